# revision 19
# baseline (speedup 1.0000x reference)
"""GCNII layer on 8 TRN2 NeuronCores (Bass/Tile).

Strategy: partition nodes (and their incoming edges, bucketed by dst) across
the 8 cores; replicate the feature table (bf16) in every core's DRAM.  Per
core, nodes are packed into 98 chunks of 128 output slots by a greedy 4-dim
bin-packer that bounds every (chunk, src-subrange) edge-cell near 512 so the
SPMD tile profile (max over cores) stays tight.  The feature table is split
into 4 sub-tables of 25000 rows so dma_gather's int16 indices reach every
row; gathers are merged into one dma_gather per (7-chunk group, subrange) to
amortize the Q7 SWDGE descriptor-generation cost, which is the kernel's
critical resource (~2ns/descriptor, engine-serial).  Scatter one-hots are
built dependency-free from constants - one broadcast-AP is_equal per chunk
on DVE - and scaled by rsqrt(deg[src]) with the work split between DVE
(broadcast-AP multiply, even chunks) and the Scalar engine (per-tile scaled
copies, odd chunks) so per-group compute stays under the ~33us gather
cadence.  The aggregation matmul runs operand-swapped (lhsT=features,
rhs=one-hot) so psum holds agg^T: the alpha initial-residual is injected as
one extra accumulating matmul (lhsT=init_chunk,
rhs=diag(alpha/(1-alpha)*sqrt(deg_dst))), and the beta/identity combine
collapses to a single matmul against the precomputed (1-beta)I + beta*W^T,
with (1-alpha)*rsqrt(deg_dst) folded into the final fused ReLU's
per-partition scale - no transposes and only one epilogue matmul per chunk.
Host-side work is integer bucketing/layout only; float math runs on device.
"""

import sys

if "/opt/trn_rl_repo" not in sys.path:
    sys.path.insert(0, "/opt/trn_rl_repo")

from contextlib import ExitStack

import ml_dtypes
import numpy as np

N, E, D, NC = 100000, 1600000, 128, 8
NPC = N // NC            # nodes per core: 12500
CHUNKS = 98              # chunks of 128 output slots per core
SLOTS = CHUNKS * 128     # padded node slots per core: 12544
ALPHA, BETA = 0.1, 0.5
NSUB = 4                 # feature-table subranges (int16 index limit)
SR = 25000               # rows per subrange
GSZ = 7                  # chunks per gather group
NG = CHUNKS // GSZ       # 14 groups
POOL_GROUPS = frozenset({2, 5, 8, 11})  # groups whose buf-scale runs on Pool

F32 = np.float32
BF16 = ml_dtypes.bfloat16


def _wrap_idx(seq):
    """dma_gather index layout: i -> [i % 16, i // 16], replicated to 128
    partitions (one copy per Q7 core)."""
    blk = seq.reshape(-1, 16).T
    return np.tile(blk, (8, 1))


def _pack_chunks(dvec, strict=93, cap=512):
    """Greedy 4-dim bin packing: nodes -> chunks so that per-(chunk,subrange)
    edge counts stay <= cap (128-slot chunks; spill chunks uncapped).
    Integer-only host work. Returns chunk_of[node]."""
    npc = dvec.shape[0]
    order = np.argsort(-dvec.sum(1), kind="stable")
    loads = np.zeros((CHUNKS, NSUB), np.int64)
    slots = np.zeros(CHUNKS, np.int64)
    assign = np.full(npc, -1, np.int64)
    spill = np.arange(CHUNKS) >= strict
    for n in order:
        w = dvec[n]
        newl = loads + w
        ok = (slots < 128) & ~spill & (newl <= cap).all(1)
        if ok.any():
            score = newl.sum(1).astype(np.float64)
            score[~ok] = -np.inf
            b = int(np.argmax(score))
        else:
            ok2 = slots < 128
            score = newl.max(1).astype(np.float64)
            score[~ok2] = np.inf
            score[~spill & ok2] += 1e6
            b = int(np.argmin(score))
        assign[n] = b
        loads[b] += w
        slots[b] += 1
    return assign, slots


def _host_prep(features, initial_features, W, src, dst, packed=True):
    """Integer-only bucketing/layout prep -> per-core device arrays."""
    src = np.ascontiguousarray(src).astype(np.int64, copy=False)
    dst = np.ascontiguousarray(dst).astype(np.int64, copy=False)
    deg = np.bincount(dst, minlength=N)
    degc = np.maximum(deg, 1).astype(F32)
    core_of = dst // NPC
    cores_tmp = []
    cnts = np.zeros((NC, CHUNKS, NSUB), np.int64)
    for c in range(NC):
        em = core_of == c
        e_src = src[em]
        e_loc = dst[em] - c * NPC
        if packed:
            dvec = np.zeros((NPC, NSUB), np.int64)
            np.add.at(dvec, (e_loc, e_src // SR), 1)
            chunk_of, nsl = _pack_chunks(dvec)
            # spread the (heavier) spill chunks across gather groups
            perm = np.argsort(np.argsort(np.arange(CHUNKS) % GSZ, kind="stable"))
            chunk_of = perm[chunk_of]
            # slot assignment within each chunk: order nodes by chunk
            slot_of = np.empty(NPC, np.int64)
            o2 = np.argsort(chunk_of, kind="stable")
            pos_in_chunk = np.arange(NPC) - np.concatenate(
                ([0], np.cumsum(np.bincount(chunk_of, minlength=CHUNKS))[:-1])
            )[chunk_of[o2]]
            slot_of[o2] = pos_in_chunk
            nodelist = np.full((CHUNKS, 128), -1, np.int64)
            nodelist[chunk_of, slot_of] = np.arange(NPC)
        else:
            ndeg = deg[c * NPC:(c + 1) * NPC]
            order = np.argsort(-ndeg, kind="stable")
            order_p = np.concatenate(
                [order, np.full(SLOTS - NPC, -1, np.int64)]
            )
            arr = order_p.reshape(128, CHUNKS)
            arr[1::2] = arr[1::2, ::-1]      # serpentine -> balanced chunks
            nodelist = arr.T.copy()          # [98,128] local node id or -1
            chunk_of = np.empty(NPC, np.int64)
            slot_of = np.empty(NPC, np.int64)
            ch = np.repeat(np.arange(CHUNKS), 128).reshape(CHUNKS, 128)
            sl = np.tile(np.arange(128), (CHUNKS, 1))
            v = nodelist >= 0
            chunk_of[nodelist[v]] = ch[v]
            slot_of[nodelist[v]] = sl[v]
        e_chunk = chunk_of[e_loc]
        e_slot = slot_of[e_loc]
        o = np.lexsort((e_src, e_chunk))     # chunk-major, src-sorted within
        e_src, e_slot, e_chunk = e_src[o], e_slot[o], e_chunk[o]
        e_sub = e_src // SR
        cnts[c] = np.bincount(
            e_chunk * NSUB + e_sub, minlength=CHUNKS * NSUB
        ).reshape(CHUNKS, NSUB)
        cores_tmp.append((e_src, e_slot, e_chunk, e_sub, nodelist))

    # uniform (SPMD) tile structure: per-(chunk,subrange) tiles = worst core
    tiles_cr = -(-cnts.max(axis=0) // 128)           # [CHUNKS, NSUB]
    K_c = tiles_cr.sum(axis=1)                       # tiles per chunk
    CB = np.zeros(CHUNKS + 1, np.int64)
    CB[1:] = np.cumsum(K_c)                          # chunk-major tile base
    TILES = int(CB[-1])
    # within-chunk r-major tile base
    kb = np.zeros((CHUNKS, NSUB), np.int64)
    kb[:, 1:] = np.cumsum(tiles_cr, axis=1)[:, :-1]
    # group/buf structure: group g covers chunks [g*GSZ,(g+1)*GSZ);
    # buf tile order within a group: (r, chunk, t)
    seg = np.zeros((NG, NSUB), np.int64)             # tiles per (group, sub)
    for g in range(NG):
        seg[g] = tiles_cr[g * GSZ:(g + 1) * GSZ].sum(axis=0)
    GT = seg.sum(axis=1)                             # tiles per group
    gtb = np.zeros(NG + 1, np.int64)
    gtb[1:] = np.cumsum(GT)                          # group buf-tile base
    rbase = np.zeros((NG, NSUB), np.int64)
    rbase[:, 1:] = np.cumsum(seg, axis=1)[:, :-1]    # sub base within group
    bufpos = np.zeros((CHUNKS, NSUB), np.int64)      # within-group tile base
    for g in range(NG):
        for r in range(NSUB):
            off = rbase[g, r]
            for c in range(g * GSZ, (g + 1) * GSZ):
                bufpos[c, r] = off
                off += tiles_cr[c, r]
    layout = dict(tiles_cr=tiles_cr, K_c=K_c, CB=CB, TILES=TILES,
                  GT=GT, gtb=gtb, rbase=rbase, bufpos=bufpos, seg=seg)

    per_core = []
    for c in range(NC):
        e_src, e_slot, e_chunk, e_sub, nodelist = cores_tmp[c]
        cnt = cnts[c]
        starts = np.zeros(CHUNKS * NSUB, np.int64)
        starts[1:] = np.cumsum(cnt.reshape(-1))[:-1]
        pos = np.arange(len(e_src)) - starts[e_chunk * NSUB + e_sub]
        t_e = pos // 128
        p_e = pos % 128
        # chunk-major global tile (for one-hot / matmul pairing)
        k_e = CB[e_chunk] + kb[e_chunk, e_sub] + t_e
        # buf-order global tile (gather order)
        g_e = e_chunk // GSZ
        gt_e = gtb[g_e] + bufpos[e_chunk, e_sub] + t_e
        rel = np.full((128, TILES), -1.0, F32)
        rel[p_e, k_e] = e_slot
        dsgB = np.ones((128, TILES), F32)
        dsgB[p_e, k_e] = degc[e_src]          # chunk-major, same order as rel
        idx_flat = np.zeros(TILES * 128, np.int16)
        idx_flat[gt_e * 128 + p_e] = (e_src - e_sub * SR).astype(np.int16)
        idx_dev = _wrap_idx(idx_flat)                 # [128, TILES*8]

        glob = np.where(nodelist >= 0, nodelist + c * NPC, -1)
        init_perm = np.zeros((SLOTS, D), F32)
        gv = glob.reshape(-1)
        init_perm[gv >= 0] = initial_features[gv[gv >= 0]]
        dcd = np.ones((CHUNKS, 128), F32)
        dcd[glob >= 0] = degc[glob[glob >= 0]]
        per_core.append(
            dict(
                eidx=np.ascontiguousarray(idx_dev),
                erel=np.ascontiguousarray(rel.astype(BF16)),
                edsg=np.ascontiguousarray(dsgB),
                dcd=np.ascontiguousarray(dcd.T),
                initp=np.ascontiguousarray(init_perm.astype(BF16)),
                glob=glob,
            )
        )
    return per_core, layout


_BUILD_CACHE = {}


def _build(layout, n_rows=N, chunks=CHUNKS, nsub=NSUB, sr=SR):
    tiles_cr = layout["tiles_cr"]
    key = (tiles_cr.tobytes(), n_rows, chunks, nsub, sr)
    if key in _BUILD_CACHE:
        return _BUILD_CACHE[key]
    import concourse.bacc as bacc
    import concourse.bass as bass  # noqa: F401
    import concourse.mybir as mybir
    import concourse.tile as tile

    f32 = mybir.dt.float32
    bf16 = mybir.dt.bfloat16
    i16 = mybir.dt.int16
    Alu = mybir.AluOpType
    Act = mybir.ActivationFunctionType

    K_c = layout["K_c"]
    CB = layout["CB"]
    TILES = layout["TILES"]
    GT = layout["GT"]
    gtb = layout["gtb"]
    rbase = layout["rbase"]
    bufpos = layout["bufpos"]
    seg = layout["seg"]
    GTmax = int(GT.max())
    Kmax = int(K_c.max())
    IDXC = TILES * 8

    nc = bacc.Bacc("TRN2", target_bir_lowering=False, num_swdge_queues=4)
    feats = nc.dram_tensor("feats", [n_rows, D], bf16, kind="ExternalInput")
    wt = nc.dram_tensor("wt", [D, D], f32, kind="ExternalInput")
    iota = nc.dram_tensor("iota", [128, 128], bf16, kind="ExternalInput")
    identb = nc.dram_tensor("identb", [128, 128], bf16, kind="ExternalInput")
    identh = nc.dram_tensor("identh", [128, 128], f32, kind="ExternalInput")
    eidx = nc.dram_tensor("eidx", [128, IDXC], i16, kind="ExternalInput")
    erel = nc.dram_tensor("erel", [128, TILES], bf16, kind="ExternalInput")
    edsg = nc.dram_tensor("edsg", [128, TILES], f32, kind="ExternalInput")
    dcd = nc.dram_tensor("dcd", [128, chunks], f32, kind="ExternalInput")
    initp = nc.dram_tensor("initp", [SLOTS, D], bf16, kind="ExternalInput")
    out = nc.dram_tensor("out", [SLOTS, D], bf16, kind="ExternalOutput")

    IA = 1.0 / (1.0 - ALPHA)

    with tile.TileContext(nc) as tc, ExitStack() as ctx:
        const = ctx.enter_context(tc.tile_pool(name="const", bufs=1))
        gpool = ctx.enter_context(tc.tile_pool(name="g", bufs=3))
        ohpool = ctx.enter_context(tc.tile_pool(name="oh", bufs=7))
        ipool = ctx.enter_context(tc.tile_pool(name="init", bufs=4))
        dpool = ctx.enter_context(tc.tile_pool(name="diag", bufs=4))
        epool = ctx.enter_context(tc.tile_pool(name="ep", bufs=4))
        opool = ctx.enter_context(tc.tile_pool(name="ob", bufs=4))
        ps_agg = ctx.enter_context(tc.tile_pool(name="psagg", bufs=6, space="PSUM"))
        ps_mm = ctx.enter_context(tc.tile_pool(name="psmm", bufs=2, space="PSUM"))

        idx_sb = const.tile([128, IDXC], i16)
        nc.sync.dma_start(out=idx_sb[:], in_=eidx[:])
        iota_sb = const.tile([128, 128], bf16)
        nc.sync.dma_start(out=iota_sb[:], in_=iota[:])
        identb_sb = const.tile([128, 128], bf16)
        nc.sync.dma_start(out=identb_sb[:], in_=identb[:])
        identh_sb = const.tile([128, 128], f32)
        nc.sync.dma_start(out=identh_sb[:], in_=identh[:])
        wt_sb = const.tile([128, 128], f32)
        nc.sync.dma_start(out=wt_sb[:], in_=wt[:])
        rel_sb = const.tile([128, TILES], bf16)
        nc.sync.dma_start(out=rel_sb[:], in_=erel[:])
        dsg_sb = const.tile([128, TILES], f32)
        nc.sync.dma_start(out=dsg_sb[:], in_=edsg[:])
        dcd_sb = const.tile([128, chunks], f32)
        nc.sync.dma_start(out=dcd_sb[:], in_=dcd[:])

        # M2 = beta*W^T + (1-beta)*I   (identh holds (1-beta)*I)
        m2_sb = const.tile([128, 128], bf16)
        nc.vector.scalar_tensor_tensor(
            m2_sb[:], wt_sb[:], BETA, identh_sb[:], Alu.mult, Alu.add
        )
        # nsrcf = rsqrt(deg[src]) chunk-major, bf16 (dsg_sb reused in place)
        nc.scalar.activation(dsg_sb[:], dsg_sb[:], Act.Sqrt)
        nc.vector.reciprocal(dsg_sb[:], dsg_sb[:])
        nsrcf_sb = const.tile([128, TILES], bf16)
        nc.scalar.activation(nsrcf_sb[:], dsg_sb[:], Act.Copy)
        # ccol = (1-alpha)*rsqrt(deg[dst]);  wcol = alpha/(1-alpha)*sqrt(deg)
        s1_sb = const.tile([128, chunks], f32)
        nc.scalar.activation(s1_sb[:], dcd_sb[:], Act.Sqrt, scale=IA * IA)
        ccol_sb = const.tile([128, chunks], f32)
        nc.vector.reciprocal(ccol_sb[:], s1_sb[:])
        w_sb = const.tile([128, chunks], f32)
        nc.scalar.activation(
            w_sb[:], dcd_sb[:], Act.Sqrt, scale=(ALPHA * IA) ** 2
        )

        q = 0
        for g in range(NG):
            buf = gpool.tile([128, GTmax * 128], bf16)
            for r in range(nsub):
                s = int(seg[g, r])
                if s == 0:
                    continue
                colbase = int(rbase[g, r])
                cb8 = (int(gtb[g]) + colbase) * 8
                nc.gpsimd.dma_gather(
                    out_ap=buf[:, colbase * 128:(colbase + s) * 128]
                    .rearrange("p (t d) -> p t d", t=s),
                    in_ap=feats[r * sr:min(n_rows, (r + 1) * sr), :],
                    idxs_ap=idx_sb[:, cb8:cb8 + s * 8],
                    num_idxs=s * 128,
                    num_idxs_reg=s * 128,
                    elem_size=D,
                    single_packet=False,
                    queue_num=q % 4,
                )
                q += 1
            for c in range(g * GSZ, (g + 1) * GSZ):
                K = int(K_c[c])
                oh = ohpool.tile([128, Kmax * 128], bf16)
                oh3 = oh[:, :K * 128].rearrange("p (t d) -> p t d", t=K)
                iota_bc = iota_sb[:].unsqueeze(1).broadcast_to([128, K, 128])
                rel_bc = (
                    rel_sb[:, int(CB[c]):int(CB[c]) + K]
                    .unsqueeze(2)
                    .broadcast_to([128, K, 128])
                )
                nc.vector.tensor_tensor(oh3, iota_bc, rel_bc, Alu.is_equal)
                if c % 2 == 0:
                    nsr_bc = (
                        nsrcf_sb[:, int(CB[c]):int(CB[c]) + K]
                        .unsqueeze(2)
                        .broadcast_to([128, K, 128])
                    )
                    nc.vector.tensor_tensor(oh3, oh3, nsr_bc, Alu.mult)
                else:
                    for k in range(K):
                        col = int(CB[c]) + k
                        nc.scalar.activation(
                            oh[:, k * 128:(k + 1) * 128],
                            oh[:, k * 128:(k + 1) * 128],
                            Act.Copy,
                            scale=dsg_sb[:, col:col + 1],
                        )
                psum = ps_agg.tile([128, 128], f32)
                k = 0
                for r in range(nsub):
                    for t in range(int(tiles_cr[c, r])):
                        bufcol = int(bufpos[c, r]) + t
                        nc.tensor.matmul(
                            psum[:],
                            lhsT=buf[:, bufcol * 128:(bufcol + 1) * 128],
                            rhs=oh[:, k * 128:(k + 1) * 128],
                            start=(k == 0),
                            stop=False,
                        )
                        k += 1
                itile = ipool.tile([128, 128], bf16)
                nc.sync.dma_start(
                    out=itile[:], in_=initp[c * 128:(c + 1) * 128, :]
                )
                diag = dpool.tile([128, 128], bf16)
                nc.scalar.activation(
                    diag[:], identb_sb[:], Act.Copy, scale=w_sb[:, c:c + 1]
                )
                nc.tensor.matmul(
                    psum[:], lhsT=itile[:], rhs=diag[:],
                    start=(k == 0), stop=True,
                )
                h3t = epool.tile([128, 128], bf16)
                nc.scalar.activation(h3t[:], psum[:], Act.Copy)
                pmm = ps_mm.tile([128, 128], f32)
                nc.tensor.matmul(
                    pmm[:], lhsT=h3t[:], rhs=m2_sb[:], start=True, stop=True
                )
                ob = opool.tile([128, 128], bf16)
                nc.scalar.activation(
                    ob[:], pmm[:], Act.Relu, scale=ccol_sb[:, c:c + 1]
                )
                nc.sync.dma_start(out=out[c * 128:(c + 1) * 128, :], in_=ob[:])

    nc.compile()
    _BUILD_CACHE[key] = nc
    return nc


def _install_ntff_shim():
    """antenv.axon_hooks is absent in this image; shim it and wire the real
    NTFF profiling hook via ctypes so trace=True works under axon."""
    import contextlib
    import ctypes
    import types

    try:
        from antenv import axon_hooks  # noqa: F401
        return
    except ImportError:
        pass
    import antenv

    mod = types.ModuleType("antenv.axon_hooks")
    _hook = [None]
    mod.set_axon_ntff_profile_hook = lambda h: _hook.__setitem__(0, h)
    mod.get_axon_ntff_profile_hook = lambda: _hook[0]
    sys.modules["antenv.axon_hooks"] = mod
    antenv.axon_hooks = mod
    try:
        lib = ctypes.CDLL("/opt/axon/libaxon_pjrt.so")
    except OSError:
        return
    if not hasattr(lib, "axon_start_nrt_profile"):
        return
    lib.axon_start_nrt_profile.argtypes = [
        ctypes.POINTER(ctypes.c_int64),
        ctypes.c_size_t,
    ]
    lib.axon_start_nrt_profile.restype = ctypes.c_int64
    lib.axon_stop_nrt_profile.argtypes = [ctypes.c_char_p]
    lib.axon_stop_nrt_profile.restype = ctypes.c_int64

    @contextlib.contextmanager
    def _hook_cm(output_dir, device_ids):
        import jax

        jax.devices()
        if device_ids:
            ids = (ctypes.c_int64 * len(device_ids))(*device_ids)
            rc = lib.axon_start_nrt_profile(ids, len(device_ids))
        else:
            rc = lib.axon_start_nrt_profile(None, 0)
        if rc != 0:
            raise RuntimeError(f"axon_start_nrt_profile rc={rc}")
        try:
            yield
        finally:
            rc = lib.axon_stop_nrt_profile(output_dir.encode())
            if rc != 0:
                print(f"WARNING: axon_stop_nrt_profile rc={rc}", flush=True)

    mod.set_axon_ntff_profile_hook(_hook_cm)


def _run(inputs, trace=False, trace_cores=None):
    from concourse import bass_utils

    if trace:
        _install_ntff_shim()
    features = np.ascontiguousarray(np.asarray(inputs["features"], dtype=F32))
    initial_features = np.ascontiguousarray(
        np.asarray(inputs["initial_features"], dtype=F32)
    )
    W = np.asarray(inputs["W"], dtype=F32)
    src = np.asarray(inputs["src"])
    dst = np.asarray(inputs["dst"])
    per_core, layout = _host_prep(features, initial_features, W, src, dst)
    nc = _build(layout)
    feats_bf = np.ascontiguousarray(features.astype(BF16))
    wt_np = np.ascontiguousarray(W.T)
    iota_np = np.ascontiguousarray(
        np.tile(np.arange(128, dtype=F32), (128, 1)).astype(BF16)
    )
    identb_np = np.eye(128, dtype=F32).astype(BF16)
    identh_np = ((1.0 - BETA) * np.eye(128)).astype(F32)
    in_maps = []
    for c in range(NC):
        pc = per_core[c]
        in_maps.append(
            dict(
                feats=feats_bf,
                wt=wt_np,
                iota=iota_np,
                identb=identb_np,
                identh=identh_np,
                eidx=pc["eidx"],
                erel=pc["erel"],
                edsg=pc["edsg"],
                dcd=pc["dcd"],
                initp=pc["initp"],
            )
        )
    res = bass_utils.run_bass_kernel_spmd(
        nc,
        in_maps,
        core_ids=list(range(NC)),
        trace=trace,
        trace_cores=trace_cores,
    )
    result = np.empty((N, D), F32)
    for c in range(NC):
        glob = per_core[c]["glob"].reshape(-1)
        oc = res.results[c]["out"]
        m = glob >= 0
        result[glob[m]] = oc[m].astype(F32)
    return result, res


def kernel(**inputs):
    return _run(inputs, trace=False)[0]


# revision 25
# speedup vs baseline: 1.0281x; 1.0281x over previous
"""GCNII layer on 8 TRN2 NeuronCores (Bass/Tile).

Strategy: partition nodes (and their incoming edges, bucketed by dst) across
the 8 cores; replicate the feature table (bf16) in every core's DRAM.  Per
core, nodes are packed into 98 chunks of 128 output slots by a greedy 4-dim
bin-packer that bounds every (chunk, src-subrange) edge-cell near 512 so the
SPMD tile profile (max over cores) stays tight.  The feature table is split
into 4 sub-tables of 25000 rows so dma_gather's int16 indices reach every
row; gathers are merged into one dma_gather per (7-chunk group, subrange) to
amortize the Q7 SWDGE descriptor-generation cost, which is the kernel's
critical resource (~2ns/descriptor, engine-serial).  Scatter one-hots are
built dependency-free from constants - one broadcast-AP is_equal per chunk
on DVE - and scaled by rsqrt(deg[src]) with the work split between DVE
(broadcast-AP multiply, even chunks) and the Scalar engine (per-tile scaled
copies, odd chunks) so per-group compute stays under the ~33us gather
cadence.  The aggregation matmul runs operand-swapped (lhsT=features,
rhs=one-hot) so psum holds agg^T: the alpha initial-residual is injected as
one extra accumulating matmul (lhsT=init_chunk,
rhs=diag(alpha/(1-alpha)*sqrt(deg_dst))), and the beta/identity combine
collapses to a single matmul against the precomputed (1-beta)I + beta*W^T,
with (1-alpha)*rsqrt(deg_dst) folded into the final fused ReLU's
per-partition scale - no transposes and only one epilogue matmul per chunk.
Host-side work is integer bucketing/layout only; float math runs on device.
"""

import sys

if "/opt/trn_rl_repo" not in sys.path:
    sys.path.insert(0, "/opt/trn_rl_repo")

from contextlib import ExitStack

import ml_dtypes
import numpy as np

N, E, D, NC = 100000, 1600000, 128, 8
NPC = N // NC            # nodes per core: 12500
CHUNKS = 98              # chunks of 128 output slots per core
SLOTS = CHUNKS * 128     # padded node slots per core: 12544
ALPHA, BETA = 0.1, 0.5
NSUB = 4                 # feature-table subranges (int16 index limit)
SR = 25000               # rows per subrange
GSZ = 7                  # chunks per gather group
NG = CHUNKS // GSZ       # 14 groups
POOL_GROUPS = frozenset({2, 5, 8, 11})  # groups whose buf-scale runs on Pool

F32 = np.float32
BF16 = ml_dtypes.bfloat16


def _wrap_idx(seq):
    """dma_gather index layout: i -> [i % 16, i // 16], replicated to 128
    partitions (one copy per Q7 core)."""
    blk = seq.reshape(-1, 16).T
    return np.tile(blk, (8, 1))


def _pack_chunks(dvec, strict=93, cap=512):
    """Greedy 4-dim bin packing: nodes -> chunks so that per-(chunk,subrange)
    edge counts stay <= cap (128-slot chunks; spill chunks uncapped).
    Integer-only host work. Returns chunk_of[node]."""
    npc = dvec.shape[0]
    order = np.argsort(-dvec.sum(1), kind="stable")
    loads = np.zeros((CHUNKS, NSUB), np.int64)
    slots = np.zeros(CHUNKS, np.int64)
    assign = np.full(npc, -1, np.int64)
    spill = np.arange(CHUNKS) >= strict
    for n in order:
        w = dvec[n]
        newl = loads + w
        ok = (slots < 128) & ~spill & (newl <= cap).all(1)
        if ok.any():
            score = newl.sum(1).astype(np.float64)
            score[~ok] = -np.inf
            b = int(np.argmax(score))
        else:
            ok2 = slots < 128
            score = newl.max(1).astype(np.float64)
            score[~ok2] = np.inf
            score[~spill & ok2] += 1e6
            b = int(np.argmin(score))
        assign[n] = b
        loads[b] += w
        slots[b] += 1
    return assign, slots


def _host_prep(features, initial_features, W, src, dst, packed=True):
    """Integer-only bucketing/layout prep -> per-core device arrays."""
    src = np.ascontiguousarray(src).astype(np.int64, copy=False)
    dst = np.ascontiguousarray(dst).astype(np.int64, copy=False)
    deg = np.bincount(dst, minlength=N)
    degc = np.maximum(deg, 1).astype(F32)
    core_of = dst // NPC
    cores_tmp = []
    cnts = np.zeros((NC, CHUNKS, NSUB), np.int64)
    for c in range(NC):
        em = core_of == c
        e_src = src[em]
        e_loc = dst[em] - c * NPC
        if packed:
            dvec = np.zeros((NPC, NSUB), np.int64)
            np.add.at(dvec, (e_loc, e_src // SR), 1)
            chunk_of, nsl = _pack_chunks(dvec)
            # slot assignment within each chunk: order nodes by chunk
            slot_of = np.empty(NPC, np.int64)
            o2 = np.argsort(chunk_of, kind="stable")
            pos_in_chunk = np.arange(NPC) - np.concatenate(
                ([0], np.cumsum(np.bincount(chunk_of, minlength=CHUNKS))[:-1])
            )[chunk_of[o2]]
            slot_of[o2] = pos_in_chunk
            nodelist = np.full((CHUNKS, 128), -1, np.int64)
            nodelist[chunk_of, slot_of] = np.arange(NPC)
        else:
            ndeg = deg[c * NPC:(c + 1) * NPC]
            order = np.argsort(-ndeg, kind="stable")
            order_p = np.concatenate(
                [order, np.full(SLOTS - NPC, -1, np.int64)]
            )
            arr = order_p.reshape(128, CHUNKS)
            arr[1::2] = arr[1::2, ::-1]      # serpentine -> balanced chunks
            nodelist = arr.T.copy()          # [98,128] local node id or -1
            chunk_of = np.empty(NPC, np.int64)
            slot_of = np.empty(NPC, np.int64)
            ch = np.repeat(np.arange(CHUNKS), 128).reshape(CHUNKS, 128)
            sl = np.tile(np.arange(128), (CHUNKS, 1))
            v = nodelist >= 0
            chunk_of[nodelist[v]] = ch[v]
            slot_of[nodelist[v]] = sl[v]
        e_chunk = chunk_of[e_loc]
        e_slot = slot_of[e_loc]
        e_sub = e_src // SR
        cnts[c] = np.bincount(
            e_chunk * NSUB + e_sub, minlength=CHUNKS * NSUB
        ).reshape(CHUNKS, NSUB)
        cores_tmp.append((e_src, e_slot, e_chunk, nodelist))

    # uniform (SPMD) tile structure: per-(chunk,subrange) tiles = worst core
    tiles0 = -(-cnts.max(axis=0) // 128)             # [CHUNKS, NSUB]
    # relabel chunks: deal heaviest chunks serpentine into the big groups,
    # lightest chunks into the small trailing groups (short pipeline drain)
    gsizes = [GSZ] * 13 + [4, 3]
    assert sum(gsizes) == CHUNKS
    gb = np.zeros(len(gsizes) + 1, np.int64)
    gb[1:] = np.cumsum(gsizes)
    NGv = len(gsizes)
    korder = np.argsort(-tiles0.sum(axis=1), kind="stable")
    grid = [[] for _ in range(NGv)]
    i = 0
    for rnd in range(GSZ):
        rng_g = range(13) if rnd % 2 == 0 else range(12, -1, -1)
        for g in rng_g:
            grid[g].append(korder[i])
            i += 1
    for g in (13, 14):
        while len(grid[g]) < gsizes[g]:
            grid[g].append(korder[i])
            i += 1
    old_of_new = np.array([c for g in range(NGv) for c in grid[g]])
    new_of_old = np.empty(CHUNKS, np.int64)
    new_of_old[old_of_new] = np.arange(CHUNKS)
    cnts = cnts[:, old_of_new]
    tiles_cr = tiles0[old_of_new]
    group_of = np.repeat(np.arange(NGv), gsizes)

    K_c = tiles_cr.sum(axis=1)                       # tiles per chunk
    CB = np.zeros(CHUNKS + 1, np.int64)
    CB[1:] = np.cumsum(K_c)                          # chunk-major tile base
    TILES = int(CB[-1])
    # within-chunk r-major tile base
    kb = np.zeros((CHUNKS, NSUB), np.int64)
    kb[:, 1:] = np.cumsum(tiles_cr, axis=1)[:, :-1]
    # group/buf structure; buf tile order within a group: (r, chunk, t)
    seg = np.zeros((NGv, NSUB), np.int64)            # tiles per (group, sub)
    for g in range(NGv):
        seg[g] = tiles_cr[gb[g]:gb[g + 1]].sum(axis=0)
    GT = seg.sum(axis=1)                             # tiles per group
    gtb = np.zeros(NGv + 1, np.int64)
    gtb[1:] = np.cumsum(GT)                          # group buf-tile base
    rbase = np.zeros((NGv, NSUB), np.int64)
    rbase[:, 1:] = np.cumsum(seg, axis=1)[:, :-1]    # sub base within group
    bufpos = np.zeros((CHUNKS, NSUB), np.int64)      # within-group tile base
    for g in range(NGv):
        for r in range(NSUB):
            off = rbase[g, r]
            for c in range(gb[g], gb[g + 1]):
                bufpos[c, r] = off
                off += tiles_cr[c, r]
    layout = dict(tiles_cr=tiles_cr, K_c=K_c, CB=CB, TILES=TILES,
                  GT=GT, gtb=gtb, rbase=rbase, bufpos=bufpos, seg=seg,
                  gb=gb)

    per_core = []
    for c in range(NC):
        e_src, e_slot, e_chunk, nodelist = cores_tmp[c]
        e_chunk = new_of_old[e_chunk]
        nodelist = nodelist[old_of_new]
        o = np.lexsort((e_src, e_chunk))     # chunk-major, src-sorted within
        e_src, e_slot, e_chunk = e_src[o], e_slot[o], e_chunk[o]
        e_sub = e_src // SR
        cnt = cnts[c]
        starts = np.zeros(CHUNKS * NSUB, np.int64)
        starts[1:] = np.cumsum(cnt.reshape(-1))[:-1]
        pos = np.arange(len(e_src)) - starts[e_chunk * NSUB + e_sub]
        t_e = pos // 128
        p_e = pos % 128
        # chunk-major global tile (for one-hot / matmul pairing)
        k_e = CB[e_chunk] + kb[e_chunk, e_sub] + t_e
        # buf-order global tile (gather order)
        g_e = group_of[e_chunk]
        gt_e = gtb[g_e] + bufpos[e_chunk, e_sub] + t_e
        rel = np.full((128, TILES), -1.0, F32)
        rel[p_e, k_e] = e_slot
        dsgB = np.ones((128, TILES), F32)
        dsgB[p_e, k_e] = degc[e_src]          # chunk-major, same order as rel
        idx_flat = np.zeros(TILES * 128, np.int16)
        idx_flat[gt_e * 128 + p_e] = (e_src - e_sub * SR).astype(np.int16)
        idx_dev = _wrap_idx(idx_flat)                 # [128, TILES*8]

        glob = np.where(nodelist >= 0, nodelist + c * NPC, -1)
        init_perm = np.zeros((SLOTS, D), F32)
        gv = glob.reshape(-1)
        init_perm[gv >= 0] = initial_features[gv[gv >= 0]]
        dcd = np.ones((CHUNKS, 128), F32)
        dcd[glob >= 0] = degc[glob[glob >= 0]]
        per_core.append(
            dict(
                eidx=np.ascontiguousarray(idx_dev),
                erel=np.ascontiguousarray(rel.astype(BF16)),
                edsg=np.ascontiguousarray(dsgB),
                dcd=np.ascontiguousarray(dcd.T),
                initp=np.ascontiguousarray(init_perm.astype(BF16)),
                glob=glob,
            )
        )
    return per_core, layout


_BUILD_CACHE = {}


def _build(layout, n_rows=N, chunks=CHUNKS, nsub=NSUB, sr=SR):
    tiles_cr = layout["tiles_cr"]
    key = (tiles_cr.tobytes(), layout["gb"].tobytes(), n_rows, chunks, nsub, sr)
    if key in _BUILD_CACHE:
        return _BUILD_CACHE[key]
    import concourse.bacc as bacc
    import concourse.bass as bass  # noqa: F401
    import concourse.mybir as mybir
    import concourse.tile as tile

    f32 = mybir.dt.float32
    bf16 = mybir.dt.bfloat16
    i16 = mybir.dt.int16
    Alu = mybir.AluOpType
    Act = mybir.ActivationFunctionType

    K_c = layout["K_c"]
    CB = layout["CB"]
    TILES = layout["TILES"]
    GT = layout["GT"]
    gtb = layout["gtb"]
    rbase = layout["rbase"]
    bufpos = layout["bufpos"]
    seg = layout["seg"]
    gb = layout["gb"]
    NGv = len(GT)
    GTmax = int(GT.max())
    Kmax = int(K_c.max())
    IDXC = TILES * 8

    nc = bacc.Bacc("TRN2", target_bir_lowering=False, num_swdge_queues=4)
    feats = nc.dram_tensor("feats", [n_rows, D], bf16, kind="ExternalInput")
    wt = nc.dram_tensor("wt", [D, D], f32, kind="ExternalInput")
    iota = nc.dram_tensor("iota", [128, 128], bf16, kind="ExternalInput")
    identb = nc.dram_tensor("identb", [128, 128], bf16, kind="ExternalInput")
    identh = nc.dram_tensor("identh", [128, 128], f32, kind="ExternalInput")
    eidx = nc.dram_tensor("eidx", [128, IDXC], i16, kind="ExternalInput")
    erel = nc.dram_tensor("erel", [128, TILES], bf16, kind="ExternalInput")
    edsg = nc.dram_tensor("edsg", [128, TILES], f32, kind="ExternalInput")
    dcd = nc.dram_tensor("dcd", [128, chunks], f32, kind="ExternalInput")
    initp = nc.dram_tensor("initp", [SLOTS, D], bf16, kind="ExternalInput")
    out = nc.dram_tensor("out", [SLOTS, D], bf16, kind="ExternalOutput")

    IA = 1.0 / (1.0 - ALPHA)

    with tile.TileContext(nc) as tc, ExitStack() as ctx:
        const = ctx.enter_context(tc.tile_pool(name="const", bufs=1))
        gpool = ctx.enter_context(tc.tile_pool(name="g", bufs=3))
        ohpool = ctx.enter_context(tc.tile_pool(name="oh", bufs=7))
        ipool = ctx.enter_context(tc.tile_pool(name="init", bufs=4))
        dpool = ctx.enter_context(tc.tile_pool(name="diag", bufs=4))
        epool = ctx.enter_context(tc.tile_pool(name="ep", bufs=4))
        opool = ctx.enter_context(tc.tile_pool(name="ob", bufs=4))
        ps_agg = ctx.enter_context(tc.tile_pool(name="psagg", bufs=6, space="PSUM"))
        ps_mm = ctx.enter_context(tc.tile_pool(name="psmm", bufs=2, space="PSUM"))

        # per-group idx tiles: the first gather only waits for its own slice
        idxg_sb = []
        for g in range(NGv):
            gB = const.tile([128, int(GT[g]) * 8], i16, name=f"idxg{g}")
            nc.sync.dma_start(
                out=gB[:],
                in_=eidx[:, int(gtb[g]) * 8:int(gtb[g] + GT[g]) * 8],
            )
            idxg_sb.append(gB)
        iota_sb = const.tile([128, 128], bf16)
        nc.sync.dma_start(out=iota_sb[:], in_=iota[:])
        identb_sb = const.tile([128, 128], bf16)
        nc.sync.dma_start(out=identb_sb[:], in_=identb[:])
        identh_sb = const.tile([128, 128], f32)
        nc.sync.dma_start(out=identh_sb[:], in_=identh[:])
        wt_sb = const.tile([128, 128], f32)
        nc.sync.dma_start(out=wt_sb[:], in_=wt[:])
        rel_sb = const.tile([128, TILES], bf16)
        nc.sync.dma_start(out=rel_sb[:], in_=erel[:])
        dsg_sb = const.tile([128, TILES], f32)
        nc.sync.dma_start(out=dsg_sb[:], in_=edsg[:])
        dcd_sb = const.tile([128, chunks], f32)
        nc.sync.dma_start(out=dcd_sb[:], in_=dcd[:])

        # M2 = beta*W^T + (1-beta)*I   (identh holds (1-beta)*I)
        m2_sb = const.tile([128, 128], bf16)
        nc.vector.scalar_tensor_tensor(
            m2_sb[:], wt_sb[:], BETA, identh_sb[:], Alu.mult, Alu.add
        )
        # nsrcf = rsqrt(deg[src]) chunk-major, bf16 (dsg_sb reused in place)
        nc.scalar.activation(dsg_sb[:], dsg_sb[:], Act.Sqrt)
        nc.vector.reciprocal(dsg_sb[:], dsg_sb[:])
        nsrcf_sb = const.tile([128, TILES], bf16)
        nc.scalar.activation(nsrcf_sb[:], dsg_sb[:], Act.Copy)
        # ccol = (1-alpha)*rsqrt(deg[dst]);  wcol = alpha/(1-alpha)*sqrt(deg)
        s1_sb = const.tile([128, chunks], f32)
        nc.scalar.activation(s1_sb[:], dcd_sb[:], Act.Sqrt, scale=IA * IA)
        ccol_sb = const.tile([128, chunks], f32)
        nc.vector.reciprocal(ccol_sb[:], s1_sb[:])
        w_sb = const.tile([128, chunks], f32)
        nc.scalar.activation(
            w_sb[:], dcd_sb[:], Act.Sqrt, scale=(ALPHA * IA) ** 2
        )

        q = 0
        for g in range(NGv):
            buf = gpool.tile([128, GTmax * 128], bf16)
            for r in range(nsub):
                s = int(seg[g, r])
                if s == 0:
                    continue
                colbase = int(rbase[g, r])
                cb8 = colbase * 8
                nc.gpsimd.dma_gather(
                    out_ap=buf[:, colbase * 128:(colbase + s) * 128]
                    .rearrange("p (t d) -> p t d", t=s),
                    in_ap=feats[r * sr:min(n_rows, (r + 1) * sr), :],
                    idxs_ap=idxg_sb[g][:, cb8:cb8 + s * 8],
                    num_idxs=s * 128,
                    num_idxs_reg=s * 128,
                    elem_size=D,
                    single_packet=False,
                    queue_num=q % 4,
                )
                q += 1
            for c in range(int(gb[g]), int(gb[g + 1])):
                K = int(K_c[c])
                oh = ohpool.tile([128, Kmax * 128], bf16)
                oh3 = oh[:, :K * 128].rearrange("p (t d) -> p t d", t=K)
                iota_bc = iota_sb[:].unsqueeze(1).broadcast_to([128, K, 128])
                rel_bc = (
                    rel_sb[:, int(CB[c]):int(CB[c]) + K]
                    .unsqueeze(2)
                    .broadcast_to([128, K, 128])
                )
                nc.vector.tensor_tensor(oh3, iota_bc, rel_bc, Alu.is_equal)
                if c % 2 == 0:
                    nsr_bc = (
                        nsrcf_sb[:, int(CB[c]):int(CB[c]) + K]
                        .unsqueeze(2)
                        .broadcast_to([128, K, 128])
                    )
                    nc.vector.tensor_tensor(oh3, oh3, nsr_bc, Alu.mult)
                else:
                    for k in range(K):
                        col = int(CB[c]) + k
                        nc.scalar.activation(
                            oh[:, k * 128:(k + 1) * 128],
                            oh[:, k * 128:(k + 1) * 128],
                            Act.Copy,
                            scale=dsg_sb[:, col:col + 1],
                        )
                psum = ps_agg.tile([128, 128], f32)
                k = 0
                for r in range(nsub):
                    for t in range(int(tiles_cr[c, r])):
                        bufcol = int(bufpos[c, r]) + t
                        nc.tensor.matmul(
                            psum[:],
                            lhsT=buf[:, bufcol * 128:(bufcol + 1) * 128],
                            rhs=oh[:, k * 128:(k + 1) * 128],
                            start=(k == 0),
                            stop=False,
                        )
                        k += 1
                itile = ipool.tile([128, 128], bf16)
                nc.sync.dma_start(
                    out=itile[:], in_=initp[c * 128:(c + 1) * 128, :]
                )
                diag = dpool.tile([128, 128], bf16)
                nc.scalar.activation(
                    diag[:], identb_sb[:], Act.Copy, scale=w_sb[:, c:c + 1]
                )
                nc.tensor.matmul(
                    psum[:], lhsT=itile[:], rhs=diag[:],
                    start=(k == 0), stop=True,
                )
                h3t = epool.tile([128, 128], bf16)
                nc.scalar.activation(h3t[:], psum[:], Act.Copy)
                pmm = ps_mm.tile([128, 128], f32)
                nc.tensor.matmul(
                    pmm[:], lhsT=h3t[:], rhs=m2_sb[:], start=True, stop=True
                )
                ob = opool.tile([128, 128], bf16)
                nc.scalar.activation(
                    ob[:], pmm[:], Act.Relu, scale=ccol_sb[:, c:c + 1]
                )
                nc.sync.dma_start(out=out[c * 128:(c + 1) * 128, :], in_=ob[:])

    nc.compile()
    _BUILD_CACHE[key] = nc
    return nc


def _install_ntff_shim():
    """antenv.axon_hooks is absent in this image; shim it and wire the real
    NTFF profiling hook via ctypes so trace=True works under axon."""
    import contextlib
    import ctypes
    import types

    try:
        from antenv import axon_hooks  # noqa: F401
        return
    except ImportError:
        pass
    import antenv

    mod = types.ModuleType("antenv.axon_hooks")
    _hook = [None]
    mod.set_axon_ntff_profile_hook = lambda h: _hook.__setitem__(0, h)
    mod.get_axon_ntff_profile_hook = lambda: _hook[0]
    sys.modules["antenv.axon_hooks"] = mod
    antenv.axon_hooks = mod
    try:
        lib = ctypes.CDLL("/opt/axon/libaxon_pjrt.so")
    except OSError:
        return
    if not hasattr(lib, "axon_start_nrt_profile"):
        return
    lib.axon_start_nrt_profile.argtypes = [
        ctypes.POINTER(ctypes.c_int64),
        ctypes.c_size_t,
    ]
    lib.axon_start_nrt_profile.restype = ctypes.c_int64
    lib.axon_stop_nrt_profile.argtypes = [ctypes.c_char_p]
    lib.axon_stop_nrt_profile.restype = ctypes.c_int64

    @contextlib.contextmanager
    def _hook_cm(output_dir, device_ids):
        import jax

        jax.devices()
        if device_ids:
            ids = (ctypes.c_int64 * len(device_ids))(*device_ids)
            rc = lib.axon_start_nrt_profile(ids, len(device_ids))
        else:
            rc = lib.axon_start_nrt_profile(None, 0)
        if rc != 0:
            raise RuntimeError(f"axon_start_nrt_profile rc={rc}")
        try:
            yield
        finally:
            rc = lib.axon_stop_nrt_profile(output_dir.encode())
            if rc != 0:
                print(f"WARNING: axon_stop_nrt_profile rc={rc}", flush=True)

    mod.set_axon_ntff_profile_hook(_hook_cm)


def _run(inputs, trace=False, trace_cores=None):
    from concourse import bass_utils

    if trace:
        _install_ntff_shim()
    features = np.ascontiguousarray(np.asarray(inputs["features"], dtype=F32))
    initial_features = np.ascontiguousarray(
        np.asarray(inputs["initial_features"], dtype=F32)
    )
    W = np.asarray(inputs["W"], dtype=F32)
    src = np.asarray(inputs["src"])
    dst = np.asarray(inputs["dst"])
    per_core, layout = _host_prep(features, initial_features, W, src, dst)
    nc = _build(layout)
    feats_bf = np.ascontiguousarray(features.astype(BF16))
    wt_np = np.ascontiguousarray(W.T)
    iota_np = np.ascontiguousarray(
        np.tile(np.arange(128, dtype=F32), (128, 1)).astype(BF16)
    )
    identb_np = np.eye(128, dtype=F32).astype(BF16)
    identh_np = ((1.0 - BETA) * np.eye(128)).astype(F32)
    in_maps = []
    for c in range(NC):
        pc = per_core[c]
        in_maps.append(
            dict(
                feats=feats_bf,
                wt=wt_np,
                iota=iota_np,
                identb=identb_np,
                identh=identh_np,
                eidx=pc["eidx"],
                erel=pc["erel"],
                edsg=pc["edsg"],
                dcd=pc["dcd"],
                initp=pc["initp"],
            )
        )
    res = bass_utils.run_bass_kernel_spmd(
        nc,
        in_maps,
        core_ids=list(range(NC)),
        trace=trace,
        trace_cores=trace_cores,
    )
    result = np.empty((N, D), F32)
    for c in range(NC):
        glob = per_core[c]["glob"].reshape(-1)
        oc = res.results[c]["out"]
        m = glob >= 0
        result[glob[m]] = oc[m].astype(F32)
    return result, res


def kernel(**inputs):
    return _run(inputs, trace=False)[0]
